# revision 1
# baseline (speedup 1.0000x reference)
"""Trainium2 Bass kernel: 3-layer autoregressive LSTM "completion" model.

Reference semantics (PyTorch LSTMCell gate order i,f,g,o):
  out0 = inputs[:, 0, :]; for t in 0..74:
    x = out; for l in 0..2: h_l, c_l = LSTMCell(x_or_h_{l-1}, h_l, c_l)
    out = hardtanh(fc(h_2)); emit out
  result[b, t, :] = out_t

Sharding: 8-way tensor parallel over the 4H gate dimension. Core g owns
H-feature slice [g*128:(g+1)*128] of every gate (512 of 4096 gate rows per
layer). All weights stay SBUF-resident in bf16. Activations are feature-major
[features, batch]; batch=512 rides the matmul free dim. After each layer's
elementwise block the 128-row h slice is AllGathered (bf16, via DRAM bounce)
into the full [1024, batch] h used by the next matmuls. fc + hardtanh are
computed redundantly on every core.
"""

import sys

if "/opt/trn_rl_repo" not in sys.path:
    sys.path.insert(0, "/opt/trn_rl_repo")

import numpy as np
import ml_dtypes

import concourse.bass as bass  # noqa: F401  (registers things)
import concourse.mybir as mybir
import concourse.tile as tile
from concourse import bacc, bass_utils

B, T, IN, H, L = 512, 75, 96, 1024, 3
NCORES = 8
HS = H // NCORES          # 128: per-core H slice
R = 4 * HS                # 512: per-core gate rows
KC = H // 128             # 8: K chunks of the hidden dim
F32 = mybir.dt.float32
BF16 = mybir.dt.bfloat16
AF = mybir.ActivationFunctionType
ALU = mybir.AluOpType

_cached = {}


def build():
    if "nc" in _cached:
        return _cached["nc"]
    nc = bacc.Bacc(
        "TRN2",
        target_bir_lowering=False,
        debug=False,
        enable_asserts=False,
        num_devices=NCORES,
    )

    # ---- DRAM I/O ----
    wx0_d = nc.dram_tensor("wx0", [IN, R], BF16, kind="ExternalInput")
    wx_d = [None] + [
        nc.dram_tensor(f"wx{l}", [128, KC * R], BF16, kind="ExternalInput")
        for l in (1, 2)
    ]
    wh_d = [
        nc.dram_tensor(f"wh{l}", [128, KC * R], BF16, kind="ExternalInput")
        for l in range(L)
    ]
    fcw_d = nc.dram_tensor("fcw", [128, KC * IN], BF16, kind="ExternalInput")
    bias_d = nc.dram_tensor("bias", [128, 4 * L], F32, kind="ExternalInput")
    fcb_d = nc.dram_tensor("fcb", [1, IN], BF16, kind="ExternalInput")
    hin_d = nc.dram_tensor("hin", [L, 128, KC * B], BF16, kind="ExternalInput")
    cin_d = nc.dram_tensor("cin", [L, 128, B], F32, kind="ExternalInput")
    x0_d = nc.dram_tensor("x0", [IN, B], BF16, kind="ExternalInput")
    out_d = nc.dram_tensor("out", [T, IN, B], F32, kind="ExternalOutput")

    CH = B // 2  # half-batch column count per stream

    with tile.TileContext(nc) as tc:
        with (
            tc.tile_pool(name="const", bufs=1) as cpool,
            tc.tile_pool(name="state", bufs=1) as spool,
            tc.tile_pool(name="work", bufs=4) as wpool,
            tc.tile_pool(name="gates", bufs=7, space="PSUM") as gpool,
            tc.tile_pool(name="fcp", bufs=1, space="PSUM") as fcpool,
            tc.tile_pool(name="dram", bufs=4, space="DRAM") as dpool,
        ):
            # ---- load constants ----
            wx0 = cpool.tile([IN, R], BF16)
            nc.sync.dma_start(wx0[:], wx0_d[:])
            wx = [None]
            for l in (1, 2):
                t_ = cpool.tile([128, KC * R], BF16, name=f"wxs{l}")
                nc.sync.dma_start(t_[:], wx_d[l][:])
                wx.append(t_)
            wh = []
            for l in range(L):
                t_ = cpool.tile([128, KC * R], BF16, name=f"whs{l}")
                nc.sync.dma_start(t_[:], wh_d[l][:])
                wh.append(t_)
            fcw = cpool.tile([128, KC * IN], BF16)
            nc.sync.dma_start(fcw[:], fcw_d[:])
            bias = cpool.tile([128, 4 * L], F32)
            nc.sync.dma_start(bias[:], bias_d[:])
            fcb = cpool.tile([1, IN], BF16)
            nc.sync.dma_start(fcb[:], fcb_d[:])
            ones = cpool.tile([1, B], BF16)
            nc.gpsimd.memset(ones[:], 1.0)

            # ---- state (two independent half-batch streams) ----
            hf = []   # hf[s][l]: full h, bf16 [128, KC*CH]
            cs = []   # cs[s][l]: cell slice, f32 [128, CH]
            xb = []   # xb[s]: current x, bf16 [IN, CH]
            for s in range(2):
                hfs, css = [], []
                for l in range(L):
                    h_ = spool.tile([128, KC * CH], BF16, name=f"hf{s}_{l}")
                    nc.sync.dma_start(
                        h_[:].rearrange("p (k c) -> p k c", k=KC),
                        hin_d[l][:].rearrange("p (k b) -> p k b", k=KC)[
                            :, :, s * CH:(s + 1) * CH
                        ],
                    )
                    hfs.append(h_)
                    c_ = spool.tile([128, CH], F32, name=f"cs{s}_{l}")
                    nc.sync.dma_start(c_[:], cin_d[l][:, s * CH:(s + 1) * CH])
                    css.append(c_)
                hf.append(hfs)
                cs.append(css)
                x_ = spool.tile([IN, CH], BF16, name=f"xb{s}")
                nc.sync.dma_start(x_[:], x0_d[:, s * CH:(s + 1) * CH])
                xb.append(x_)

            # ---- helpers ----
            # q is the weight-layout gate index: 0=i 1=f 2=g 3=o.
            QORDER = (0, 1, 2, 3)

            def rec_mms(l, s, ps):
                """Recurrent-term matmuls for (layer l, stream s)."""
                for q in QORDER:
                    for k in range(KC):
                        nc.tensor.matmul(
                            ps[q][:],
                            wh[l][:, k * R + q * 128:k * R + (q + 1) * 128],
                            hf[s][l][:, k * CH:(k + 1) * CH],
                            start=(k == 0),
                            stop=False,
                        )

            def x_mms(l, s, ps):
                """Input-term matmuls (x for l=0, h_{l-1} otherwise)."""
                if l == 0:
                    for q in QORDER:
                        nc.tensor.matmul(
                            ps[q][:],
                            wx0[:, q * 128:(q + 1) * 128],
                            xb[s][:],
                            start=False,
                            stop=True,
                        )
                else:
                    for q in QORDER:
                        for k in range(KC):
                            nc.tensor.matmul(
                                ps[q][:],
                                wx[l][:, k * R + q * 128:k * R + (q + 1) * 128],
                                hf[s][l - 1][:, k * CH:(k + 1) * CH],
                                start=False,
                                stop=(k == KC - 1),
                            )

            def ew_and_allgather(l, s, ps):
                """Gates, cell update, h slice, AllGather into hf[s][l]."""
                sf = wpool.tile([128, CH], F32, tag="sf", name=f"sf{s}_{l}")
                si = wpool.tile([128, CH], F32, tag="si", name=f"si{s}_{l}")
                tg = wpool.tile([128, CH], F32, tag="tg", name=f"tg{s}_{l}")
                so = wpool.tile([128, CH], F32, tag="so", name=f"so{s}_{l}")
                t1 = wpool.tile([128, CH], F32, tag="t1", name=f"t1_{s}_{l}")
                t2 = wpool.tile([128, CH], F32, tag="t2", name=f"t2_{s}_{l}")
                tc_ = wpool.tile([128, CH], F32, tag="tc", name=f"tc{s}_{l}")
                hb = wpool.tile([128, CH], BF16, tag="hb", name=f"hb{s}_{l}")
                nc.scalar.activation(si[:], ps[0][:], AF.Sigmoid, bias=bias[:, 4 * l + 0:4 * l + 1])
                nc.scalar.activation(sf[:], ps[1][:], AF.Sigmoid, bias=bias[:, 4 * l + 1:4 * l + 2])
                nc.scalar.activation(tg[:], ps[2][:], AF.Tanh, bias=bias[:, 4 * l + 2:4 * l + 3])
                nc.scalar.activation(so[:], ps[3][:], AF.Sigmoid, bias=bias[:, 4 * l + 3:4 * l + 4])
                nc.vector.tensor_mul(t1[:], sf[:], cs[s][l][:])
                nc.vector.tensor_mul(t2[:], si[:], tg[:])
                nc.vector.tensor_add(cs[s][l][:], t1[:], t2[:])
                nc.scalar.activation(tc_[:], cs[s][l][:], AF.Tanh)
                nc.vector.tensor_mul(hb[:], so[:], tc_[:])

                ci = dpool.tile([128, CH], BF16, tag="ccin", name=f"ci{s}_{l}")
                nc.sync.dma_start(ci[:], hb[:])
                co = dpool.tile(
                    [NCORES * 128, CH], BF16, tag="ccout",
                    addr_space="Shared", name=f"co{s}_{l}",
                )
                nc.gpsimd.collective_compute(
                    "AllGather",
                    ALU.bypass,
                    replica_groups=[list(range(NCORES))],
                    ins=[ci[:]], outs=[co[:]],
                )
                for k in range(KC):
                    nc.sync.dma_start(
                        hf[s][l][:, k * CH:(k + 1) * CH],
                        co[k * 128:(k + 1) * 128, :],
                    )

            def new_gates(t, l, s):
                return [
                    gpool.tile([128, CH], F32, tag="gates", name=f"g{t}_{l}_{s}_{q}")
                    for q in range(4)
                ]

            def fc_phase(t, s):
                pf = fcpool.tile([IN, CH], F32, tag="fc", name=f"fc{t}_{s}")
                nc.tensor.matmul(pf[:], fcb[:], ones[:, :CH], start=True, stop=False)
                for k in range(KC):
                    nc.tensor.matmul(
                        pf[:],
                        fcw[:, k * IN:(k + 1) * IN],
                        hf[s][2][:, k * CH:(k + 1) * CH],
                        start=False,
                        stop=(k == KC - 1),
                    )
                if t < T - 1:
                    nc.vector.tensor_scalar(
                        xb[s][:], pf[:], 1.0, -1.0, op0=ALU.min, op1=ALU.max
                    )
                xc = wpool.tile([IN, CH], F32, tag="xc", name=f"xc{t}_{s}")
                nc.vector.tensor_scalar(
                    xc[:], pf[:], 1.0, -1.0, op0=ALU.min, op1=ALU.max
                )
                nc.sync.dma_start(out_d[t][:, s * CH:(s + 1) * CH], xc[:])

            # ---- time loop (fully unrolled, two phase-offset streams) ----
            # Stream B's matmuls fill the PE while stream A's AllGather is in
            # flight and vice versa; independent AllGathers pipeline on the
            # collective hardware.
            for t in range(T):
                for l in range(L):
                    for s in range(2):
                        psl = new_gates(t, l, s)
                        rec_mms(l, s, psl)
                        x_mms(l, s, psl)
                        ew_and_allgather(l, s, psl)
                for s in range(2):
                    fc_phase(t, s)

    nc.compile()
    _cached["nc"] = nc
    return nc


def make_in_maps(inputs, hiddens, cells, Ws, fc_w, fc_b):
    """Ws: list of (Wih, Whh, bih, bhh) per layer. Returns per-core input maps."""
    bf = ml_dtypes.bfloat16

    def to_lhsT_layout(wt):  # wt: [1024, R] (already transposed slice)
        return np.ascontiguousarray(
            wt.reshape(KC, 128, R).transpose(1, 0, 2).reshape(128, KC * R)
        )

    # full initial h, feature-major, chunked: [L, 128, KC*B]
    hin = np.ascontiguousarray(
        hiddens.transpose(0, 2, 1)  # [L, H, B]
        .reshape(L, KC, 128, B)
        .transpose(0, 2, 1, 3)
        .reshape(L, 128, KC * B)
    ).astype(bf)
    x0 = np.ascontiguousarray(inputs[:, 0, :].T).astype(bf)
    fcw = np.ascontiguousarray(
        fc_w.T.reshape(KC, 128, IN).transpose(1, 0, 2).reshape(128, KC * IN)
    ).astype(bf)
    fcb = fc_b.reshape(1, IN).astype(bf)

    in_maps = []
    for g in range(NCORES):
        rows = np.concatenate(
            [np.arange(q * H + g * HS, q * H + (g + 1) * HS) for q in range(4)]
        )
        m = {"hin": hin, "x0": x0, "fcw": fcw, "fcb": fcb}
        bias_pack = np.zeros((128, 4 * L), np.float32)
        for l, (Wih, Whh, bih, bhh) in enumerate(Ws):
            wx_sl = np.ascontiguousarray(Wih[rows].T)  # [in_dim, R]
            wh_sl = np.ascontiguousarray(Whh[rows].T)  # [1024, R]
            if l == 0:
                m["wx0"] = wx_sl.astype(bf)
            else:
                m[f"wx{l}"] = to_lhsT_layout(wx_sl).astype(bf)
            m[f"wh{l}"] = to_lhsT_layout(wh_sl).astype(bf)
            bsl = (bih + bhh)[rows]  # [R]
            bias_pack[:, 4 * l:4 * (l + 1)] = bsl.reshape(4, HS).T
        m["bias"] = bias_pack
        m["cin"] = np.ascontiguousarray(
            cells[:, :, g * HS:(g + 1) * HS].transpose(0, 2, 1)
        ).astype(np.float32)
        in_maps.append(m)
    return in_maps


def kernel(**inputs):
    nc = build()
    Ws = [
        (
            np.asarray(inputs[f"Wih{l}"], np.float32),
            np.asarray(inputs[f"Whh{l}"], np.float32),
            np.asarray(inputs[f"bih{l}"], np.float32),
            np.asarray(inputs[f"bhh{l}"], np.float32),
        )
        for l in range(L)
    ]
    in_maps = make_in_maps(
        np.asarray(inputs["inputs"], np.float32),
        np.asarray(inputs["hiddens"], np.float32),
        np.asarray(inputs["cells"], np.float32),
        Ws,
        np.asarray(inputs["fc_w"], np.float32),
        np.asarray(inputs["fc_b"], np.float32),
    )
    res = bass_utils.run_bass_kernel_spmd(
        nc, in_maps, core_ids=list(range(NCORES))
    )
    out = res.results[0]["out"]  # [T, IN, B] f32
    return np.ascontiguousarray(out.transpose(2, 0, 1)).astype(np.float32)

